# revision 21
# baseline (speedup 1.0000x reference)
"""Trainium2 Bass kernel for nn_Conv2dKan (KAN 3x3 conv, Hermite basis 8 + silu residual).

Full-input contract: kernel(x, w_b, w_s, c) -> [16, 128, 32, 32] fp32.

Math:
  out[b,o,l] = sum_{i,k,a} (w_s*c)[i,o,k,a] * H_a(xw[b,i,k,l])
             + sum_{i,k}   w_b[i,o,k]      * silu(xw[b,i,k,l])
  where xw = 3x3 unfold of x with zero padding 1.

Kernel strategy (v2):
  - Hermite basis folded into monomials x^m host-side (exact); the m=0 term
    becomes a per-channel output bias.
  - Feature PAIRS on partition halves, ordered so the first pair is ready
    almost immediately: j=0 (x, silu), j=1 (x^2,x^3), j=2 (x^4,x^5),
    j=3 (x^6,x^7). The silu (residual, dominant term) and x need only a
    sigmoid + mul, so the real matmul stream starts ~3us in instead of ~12.
  - Matmul loop: two passes over spatial halves (nh=0 then nh=1); within a
    pass, (j,k) outer with both images' matmuls back-to-back per weight
    (2x weight reuse), PSUM banks per image. nh=0 outputs drain + DMA while
    nh=1 matmuls run, hiding the output tail.
  - ACT does only sigmoid (table preloaded on dummy data during input DMA)
    + the even-power bf16 casts; DVE does the fp32 power chain + odd powers;
    GPSIMD (pool) does border memsets + one tail bias-add.
  - Data parallel over batch: 16 images / 8 cores.
"""

import numpy as np
import ml_dtypes

import concourse.bacc as bacc
import concourse.mybir as mybir
import concourse.tile as tile
from concourse.bass_utils import run_bass_kernel_spmd

F32 = mybir.dt.float32
BF16 = mybir.dt.bfloat16

B, CIN, H, W = 16, 64, 32, 32
COUT = 128
K2 = 9          # 3x3 taps
BASIS = 8       # Hermite orders 0..7
NCORES = 8
IMGS_PER_CORE = B // NCORES  # 2
HP, WP = H + 2, W + 2        # padded 34x34
LP = HP * WP                 # 1156
L = H * W                    # 1024
NHALF = 512                  # psum free dim (half the image)
NPAIR = 4  # feature pairs: (x,silu) (x2,x3) (x4,x5) (x6,x7)

_CACHE = {}


def _hermite_coeff_matrix():
    """C[a, m] = coefficient of x^m in physicists' Hermite H_a, a,m in 0..7."""
    C = np.zeros((BASIS, BASIS), dtype=np.float64)
    C[0, 0] = 1.0
    C[1, 1] = 2.0
    for n in range(1, BASIS - 1):
        # H_{n+1} = 2 x H_n - 2 n H_{n-1}
        C[n + 1, 1:] += 2.0 * C[n, :-1]
        C[n + 1, :] -= 2.0 * n * C[n - 1, :]
    return C


def _build_program():
    """Build + compile the per-core Bass program (cached per process)."""
    if "nc" in _CACHE:
        return _CACHE["nc"]

    nc = bacc.Bacc("TRN2", target_bir_lowering=False, debug=False,
                   num_devices=NCORES)

    x_in = nc.dram_tensor("x_in", [IMGS_PER_CORE, CIN, H, W], F32,
                          kind="ExternalInput").ap()
    # weight layout: [p, (j*K2 + k)*COUT + o]; p<64 -> feature f0(j) chan p,
    # p>=64 -> feature f1(j) chan p-64
    w_in = nc.dram_tensor("w_in", [128, NPAIR * K2 * COUT], BF16,
                          kind="ExternalInput").ap()
    b_in = nc.dram_tensor("b_in", [COUT, 1], F32, kind="ExternalInput").ap()
    y_out = nc.dram_tensor("y_out", [IMGS_PER_CORE, COUT, L], F32,
                           kind="ExternalOutput").ap()

    with tile.TileContext(nc) as tc:
        _kernel_body(nc, tc, x_in, w_in, b_in, y_out)

    nc.compile()
    _CACHE["nc"] = nc
    return nc


def _kernel_body(nc, tc, x_in, w_in, b_in, y_out):
    SILU = mybir.ActivationFunctionType.Silu
    with (
        tc.tile_pool(name="wpool", bufs=1) as wpool,
        tc.tile_pool(name="fpool", bufs=1) as fpool,
        tc.tile_pool(name="iopool", bufs=4) as iopool,
        tc.tile_pool(name="psum", bufs=4, space="PSUM") as ppool,
    ):
        # --- PE warmup scratch (memset first so warm MMs start immediately)
        warm_w = wpool.tile([128, COUT], BF16, name="warm_w")
        nc.gpsimd.memset(warm_w, 0.0)
        warm_f = wpool.tile([128, NHALF], BF16, name="warm_f")
        nc.gpsimd.memset(warm_f, 0.0)
        ps_warm = ppool.tile([COUT, NHALF], F32, name="ps_warm", tag="warm",
                             bufs=1)

        # --- ACT silu table preload on dummy data (overlaps input DMA)
        sig_dummy = fpool.tile([1, 16], F32, name="sig_dummy")
        nc.scalar.activation(sig_dummy, warm_w[0:1, 0:16], SILU)

        # --- input stage DMAs first (pair-0 features gate the MM stream):
        # r0 lower half (x copy dep), r0 upper (silu dep), r1 upper, r1 lower
        st_ = []
        for r in range(IMGS_PER_CORE):
            st_.append(fpool.tile([128, L], F32, name=f"stage{r}"))
        xf = [x_in[r].rearrange("c h w -> c (h w)")
              for r in range(IMGS_PER_CORE)]
        w_0 = wpool.tile([128, K2 * COUT], BF16, name="w_0")
        # order: silu r0 dep, x-copy r0 dep, first taps of j=0 weights, r1
        nc.sync.dma_start(st_[0][CIN:, :], xf[0])
        nc.sync.dma_start(st_[0][:CIN, :], xf[0])
        nc.sync.dma_start(w_0[:, 0:3 * COUT], w_in[:, 0:3 * COUT])
        nc.sync.dma_start(st_[1][CIN:, :], xf[1])
        nc.sync.dma_start(st_[1][:CIN, :], xf[1])
        nc.sync.dma_start(w_0[:, 3 * COUT:], w_in[:, 3 * COUT:K2 * COUT])

        wt = [None] * NPAIR
        wt[0] = w_0
        for j in range(1, NPAIR):
            w_j = wpool.tile([128, K2 * COUT], BF16, name=f"w_{j}")
            nc.sync.dma_start(w_j, w_in[:, j * K2 * COUT:(j + 1) * K2 * COUT])
            wt[j] = w_j
        bias = wpool.tile([COUT, 1], F32, name="bias")
        nc.sync.dma_start(bias, b_in)

        # ~9 warm MMs: release the HAM clock gate while features build
        for _ in range(9):
            nc.tensor.matmul(ps_warm, warm_w, warm_f, start=True, stop=True)

        # --- padded bf16 feature tiles; borders zero (all features vanish at
        # x=0 once the constant basis term is folded out). One whole-tile
        # memset each (gpsimd dispatch is ~1us/instr, so fewer+bigger ops);
        # pair-0 tiles go on DVE so they're ready before the first interior
        # writes, the rest on gpsimd off the critical path.
        Bt = [[None] * NPAIR for _ in range(IMGS_PER_CORE)]
        Bi = [[None] * NPAIR for _ in range(IMGS_PER_CORE)]  # interior views
        for j in range(NPAIR):
            for r in range(IMGS_PER_CORE):
                b_j = fpool.tile([128, LP], BF16, name=f"b{j}_{r}")
                b3v = b_j.rearrange("p (h w) -> p h w", w=WP)
                if j == 0:
                    # pair-0 borders on DVE: small strip memsets, no deps,
                    # done long before the first interior write
                    nc.vector.memset(b3v[:, 0:1, :], 0.0)
                    nc.vector.memset(b3v[:, H + 1:H + 2, :], 0.0)
                    nc.vector.memset(b3v[:, 1:H + 1, 0:1], 0.0)
                    nc.vector.memset(b3v[:, 1:H + 1, W + 1:W + 2], 0.0)
                else:
                    nc.gpsimd.memset(b_j, 0.0)
                Bt[r][j] = b_j
                Bi[r][j] = b3v[:, 1:H + 1, 1:W + 1]

        # --- features. pair 0 = (x, silu) straight off the stage; r0's whole
        # chain before r1's (r1's matmul block runs ~15us later). Power chain
        # d=x^2, p4=x^4, p6=x^6 in fp32 on DVE; odd powers bf16-out DVE muls;
        # even-power casts on ACT (its silu work is done by then).
        # image 0's pair-0 writes split at interior row 18: the nh=0 matmuls
        # read only rows 0..17, so they unblock after the first half
        HS = 18
        nc.vector.tensor_copy(Bi[0][0][:CIN, :HS, :],
                              st_[0][:CIN, :HS * W])                     # x
        nc.scalar.activation(Bi[0][0][CIN:, :HS, :],
                             st_[0][CIN:, :HS * W], SILU)
        nc.vector.tensor_copy(Bi[0][0][:CIN, HS:, :],
                              st_[0][:CIN, HS * W:])
        nc.scalar.activation(Bi[0][0][CIN:, HS:, :],
                             st_[0][CIN:, HS * W:], SILU)
        nc.scalar.activation(Bi[1][0][CIN:], st_[1][CIN:, :], SILU)
        d_, p4_, p6_ = [None, None], [None, None], [None, None]
        for r in range(IMGS_PER_CORE):
            d = fpool.tile([128, L], F32, name=f"d{r}")
            nc.vector.tensor_mul(d, st_[r], st_[r])                      # x^2
            d_[r] = d
            nc.vector.tensor_mul(Bi[r][1][CIN:], st_[r][CIN:, :], d[CIN:, :])
            nc.scalar.copy(Bi[r][1][:CIN], d[:CIN, :])                   # x^2
            p4 = fpool.tile([128, L], F32, name=f"p4_{r}")
            nc.vector.tensor_mul(p4, d, d)                               # x^4
            p4_[r] = p4
            nc.vector.tensor_mul(Bi[r][2][CIN:], st_[r][CIN:, :], p4[CIN:, :])
            nc.scalar.copy(Bi[r][2][:CIN], p4[:CIN, :])                  # x^4
            p6 = fpool.tile([128, L], F32, name=f"p6_{r}")
            nc.vector.tensor_mul(p6, p4, d)                              # x^6
            p6_[r] = p6
            nc.vector.tensor_mul(Bi[r][3][CIN:], st_[r][CIN:, :], p6[CIN:, :])
            nc.scalar.copy(Bi[r][3][:CIN], p6[:CIN, :])                  # x^6
            if r == 0:
                nc.vector.tensor_copy(Bi[1][0][:CIN], st_[1][:CIN, :])   # x r1

        # --- conv: r-major passes; within a pass j-major with both spatial
        # halves interleaved per tap (one j-block = 18 MMs ~ 3.8us, pacing
        # that keeps ahead of the feature chain). r0's drains + output DMAs
        # overlap r1's matmuls; only r1's tail is exposed.
        n_acc = NPAIR * K2
        for r in range(IMGS_PER_CORE):
            psums = [ppool.tile([COUT, NHALF], F32, name=f"ps{nh}_{r}",
                                tag="ps")
                     for nh in range(2)]
            nhs = (0, 1) if r == 0 else (1, 0)  # last pass: nh1 drains first
            for j in range(NPAIR):
                g3 = Bt[r][j].rearrange("p (h w) -> p h w", w=WP)
                # last image, last j-block: finish nh1 entirely first so its
                # drain + output DMA overlap nh0's final 9 matmuls
                split = (r == IMGS_PER_CORE - 1 and j == NPAIR - 1)
                korder = ([(k, (nh,)) for nh in nhs for k in range(K2)]
                          if split else [(k, nhs) for k in range(K2)])
                for k, knhs in korder:
                    kh, kw = divmod(k, 3)
                    cnt = j * K2 + k
                    lhsT = wt[j][:, k * COUT:(k + 1) * COUT]
                    for nh in knhs:
                        rhs = g3[:, nh * 16 + kh: nh * 16 + kh + 16,
                                 kw: kw + W]
                        nc.tensor.matmul(psums[nh], lhsT, rhs,
                                         start=(cnt == 0),
                                         stop=(cnt == n_acc - 1))
            for nh in nhs:
                o_sb = iopool.tile([COUT, NHALF], F32, name=f"osb{nh}_{r}",
                                   tag="osb")
                nc.vector.tensor_scalar(o_sb, psums[nh], bias, None,
                                        op0=mybir.AluOpType.add)
                nc.sync.dma_start(y_out[r, :, nh * NHALF:(nh + 1) * NHALF],
                                  o_sb)


def _prepare_host_inputs(x, w_b, w_s, c):
    """Fold Hermite->monomial transform into weights; build per-core inputs."""
    x = np.asarray(x, dtype=np.float32)
    w_b64 = np.asarray(w_b, dtype=np.float64)[..., 0]          # [i,o,k]
    w_s64 = np.asarray(w_s, dtype=np.float64)[..., 0]          # [i,o,k]
    c64 = np.asarray(c, dtype=np.float64)[:, :, :, 0, :]       # [i,o,k,a]

    cw = w_s64[..., None] * c64                                # [i,o,k,a]
    C = _hermite_coeff_matrix()                                # [a,m]
    w_mono = np.einsum("ioka,am->iokm", cw, C)                 # [i,o,k,m]

    bias = w_mono[..., 0].sum(axis=(0, 2)).astype(np.float32)  # [o]

    # w_host[p, (j*K2 + k)*COUT + o]; pair order (x,silu)(x2,x3)(x4,x5)(x6,x7)
    pair_feats = [(1, None), (2, 3), (4, 5), (6, 7)]  # None -> silu (w_b)
    w_host = np.zeros((128, NPAIR * K2 * COUT), dtype=np.float64)
    for j, (m0, m1) in enumerate(pair_feats):
        f0 = w_mono[:, :, :, m0]                               # [i,o,k]
        f1 = w_b64 if m1 is None else w_mono[:, :, :, m1]
        blk0 = np.transpose(f0, (0, 2, 1)).reshape(CIN, K2 * COUT)
        blk1 = np.transpose(f1, (0, 2, 1)).reshape(CIN, K2 * COUT)
        w_host[:CIN, j * K2 * COUT:(j + 1) * K2 * COUT] = blk0
        w_host[CIN:, j * K2 * COUT:(j + 1) * K2 * COUT] = blk1
    w_host = w_host.astype(ml_dtypes.bfloat16)

    in_maps = []
    for core in range(NCORES):
        xs = x[core * IMGS_PER_CORE:(core + 1) * IMGS_PER_CORE]
        in_maps.append({
            "x_in": np.ascontiguousarray(xs),
            "w_in": w_host,
            "b_in": bias.reshape(COUT, 1),
        })
    return in_maps, w_host.astype(np.float64), bias


def _spot_reference(x, w_host64, bias, b_idx, n_out=16):
    """Numpy mini-reference for one image, first n_out channels (kernel math)."""
    xp = np.zeros((CIN, HP, WP), dtype=np.float64)
    xp[:, 1:H + 1, 1:W + 1] = x[b_idx].astype(np.float64)
    silu = xp / (1.0 + np.exp(-xp))
    pair_vals = [(xp, silu), (xp ** 2, xp ** 3), (xp ** 4, xp ** 5),
                 (xp ** 6, xp ** 7)]
    feats = [np.concatenate([f0, f1], axis=0) for f0, f1 in pair_vals]
    out = np.tile(bias[:n_out, None].astype(np.float64), (1, L))  # [n_out, L]
    for j in range(NPAIR):
        for k in range(K2):
            kh, kw = divmod(k, 3)
            win = feats[j][:, kh:kh + H, kw:kw + W].reshape(128, L)
            wk = w_host64[:, (j * K2 + k) * COUT:(j * K2 + k) * COUT + n_out]
            out += wk.T @ win
    return out  # [n_out, L] float64


def kernel(x, w_b, w_s, c):
    nc = _build_program()
    in_maps, w_host64, bias = _prepare_host_inputs(x, w_b, w_s, c)
    x = np.asarray(x, dtype=np.float32)

    last_err = None
    for _attempt in range(3):
        try:
            res = run_bass_kernel_spmd(nc, in_maps, core_ids=list(range(NCORES)))
        except Exception as e:  # transient tunnel/device failures
            last_err = e
            continue
        out = np.concatenate(
            [res.results[core]["y_out"].reshape(IMGS_PER_CORE, COUT, H, W)
             for core in range(NCORES)], axis=0).astype(np.float32)
        # guard against transient device garbage: spot-check 1 image per core
        ok = np.isfinite(out).all()
        if ok:
            for core in range(NCORES):
                b_idx = core * IMGS_PER_CORE
                ref = _spot_reference(x, w_host64, bias, b_idx)
                got = out[b_idx, :16].reshape(16, L).astype(np.float64)
                err = np.linalg.norm(got - ref) / (np.linalg.norm(ref) + 1e-30)
                if not np.isfinite(err) or err > 3e-2:
                    ok = False
                    break
        if ok:
            return out
    raise RuntimeError(
        f"kernel: device output failed spot-check after 3 attempts ({last_err})")
